# revision 30
# baseline (speedup 1.0000x reference)
"""Trainium2 Bass/Tile kernel: squared-L2 distance to prototypes (vq codebook).

D[n, p] = |x_n|^2 + |w_p|^2 - 2 * x @ w^T
  x: [65536, 512] f32, w: [512, 512] f32 -> D: [65536, 512] f32

Sharding: x rows split 8 ways across NeuronCores (data parallel), w replicated.
Per-core work: [8192, 512] @ [512, 512]^T plus rank-1 row/col corrections.

Per-core pipeline (all static python loops, Tile framework), measured at
~122 us/core on TRN2 (8 cores in parallel; DMA-bound, ~330 GB/s aggregate):
  - preprocess: w -> wT scaled by -2 (PE transposes), wsq broadcast tile
  - per 128-row tile of x:
      ACT: xsq = row-sum of x^2 (Square activation + accumulator)
      PE : 4x 128x128 transposes of the x tile into PSUM (fp32r)
      ACT/DVE: copy PSUM -> SBUF (xT), alternating engines to balance
      PE : 4 accumulating matmuls vs -2*wT (fp32r) -> PSUM = -2*cross
      DVE: out = (psum + xsq) + wsq_bcast in one scalar_tensor_tensor op
      DMA: 2MB chunk loads (contiguous 16KB/partition descriptors via
           (c p t) row mapping), 1MB half-chunk stores on the second
           HWDGE ring

float32r is fp32 bit-layout run in the PE's fast (tf32-like) matmul mode:
1 cycle/row vs 4 for exact fp32. Accumulation stays fp32 in PSUM. Walrus
requires every producer feeding an fp32r matmul to emit fp32r-rounded
output, so x/w DRAM tensors and all PE-operand tiles are declared float32r
(external inputs and DVE/ACT/memset writes all count as rounded).
"""

from contextlib import ExitStack

import numpy as np

import concourse.bass as bass
import concourse.mybir as mybir
import concourse.tile as tile
from concourse.masks import make_identity


def split_multi_waits(nc: bass.Bass, limit: int = 1) -> int:
    """Walrus codegen on this stack encodes at most one sync-wait per TPB
    instruction ("Too many sync wait commands"). Hoist excess waits onto
    same-engine InstNoOp carriers inserted just before the instruction —
    engine queues run in order, so stalling the nop stalls the instruction."""
    n_split = 0
    for block in nc.m.functions[0].blocks:
        new_insts = []
        for inst in block.instructions:
            si = inst.sync_info
            if si is not None and si.on_wait and len(si.on_wait) > limit:
                waits = list(si.on_wait)
                extra, keep = waits[: len(waits) - limit], waits[len(waits) - limit :]
                for i in range(0, len(extra), limit):
                    nop = mybir.InstNoOp(
                        name=nc.get_next_instruction_name(),
                        engine=inst.engine,
                        ins=[],
                        outs=[],
                        sync_info=mybir.SyncInfo(
                            on_wait=extra[i : i + limit], on_update=[]
                        ),
                        bass_nofuse=True,
                    )
                    nc.register_instruction(nop)
                    new_insts.append(nop)
                    n_split += 1
                inst.sync_info = mybir.SyncInfo(
                    on_wait=keep, on_update=list(si.on_update)
                )
            new_insts.append(inst)
        block.instructions[:] = new_insts
    return n_split

N, F, P = 65536, 512, 512
NCORES = 8
N_LOC = N // NCORES  # 8192 rows per core

FP32 = mybir.dt.float32
FP32R = mybir.dt.float32r
AF = mybir.ActivationFunctionType
ALU = mybir.AluOpType

KS = F // 128  # 4 contraction slices
PB = P // 128  # 4 prototype row-blocks


def drop_sem_range_clear(nc: bass.Bass) -> int:
    """This stack's walrus rejects the EVENT_SEMAPHORE_RANGE_CLEAR InstISA
    ("ISA wrong length") that bass emits as kernel-tail semaphore cleanup.
    Drop it; re-execution correctness is validated by running twice."""
    n = 0
    for block in nc.m.functions[0].blocks:
        keep = []
        for inst in block.instructions:
            if (
                isinstance(inst, mybir.InstISA)
                and getattr(inst, "op_name", None) == "EVENT_SEMAPHORE_RANGE_CLEAR"
            ):
                n += 1
                continue
            keep.append(inst)
        block.instructions[:] = keep
    return n


def build_bass(n_loc: int = N_LOC, chunk: int = 8) -> bass.Bass:
    """Build the per-core Bass program. n_loc = rows of x on this core."""
    tiles = n_loc // 128
    assert tiles % chunk == 0
    nchunks = tiles // chunk

    nc = bass.Bass("TRN2", target_bir_lowering=False, debug=False)
    x = nc.dram_tensor("inputs", [n_loc, F], FP32R, kind="ExternalInput").ap()
    w = nc.dram_tensor("w", [P, F], FP32R, kind="ExternalInput").ap()
    out = nc.dram_tensor("out", [n_loc, P], FP32, kind="ExternalOutput").ap()

    # Partition p of chunk c holds rows [c*128*chunk + p*chunk, +chunk): each
    # partition's slice is `chunk` consecutive rows = one contiguous DRAM
    # extent (16KB at chunk=8), so the DMA emits 128 big descriptors instead
    # of 128*chunk 2KB ones. Row->tile mapping changes but rows are
    # independent, so compute is unaffected as long as out uses the same map.
    x_r = x.rearrange("(c p t) f -> c p t f", p=128, t=chunk)
    out_r = out.rearrange("(c p t) q -> c p t q", p=128, t=chunk)
    w_r = w.rearrange("(b p) f -> p b f", p=128)

    # chunk schedule: taper the last full chunk into two half chunks so the
    # end-of-pipeline drain (compute+store of the final chunk) is halved
    sched = [(x_r[c], out_r[c], chunk) for c in range(nchunks)]
    if nchunks >= 4 and chunk % 2 == 0:
        h = chunk // 2
        rows = 128 * chunk
        tail0 = (nchunks - 1) * rows
        xt_r = x[tail0:, :].rearrange("(c p t) f -> c p t f", p=128, t=h)
        ot_r = out[tail0:, :].rearrange("(c p t) q -> c p t q", p=128, t=h)
        sched = sched[:-1] + [(xt_r[0], ot_r[0], h), (xt_r[1], ot_r[1], h)]

    with tile.TileContext(nc) as tc, ExitStack() as ctx:
        const = ctx.enter_context(tc.tile_pool(name="const", bufs=1))
        wpool = ctx.enter_context(tc.tile_pool(name="w", bufs=1))
        xpool = ctx.enter_context(tc.tile_pool(name="x", bufs=3))
        opool = ctx.enter_context(tc.tile_pool(name="o", bufs=3))
        xtpool = ctx.enter_context(tc.tile_pool(name="xt", bufs=4))
        spool = ctx.enter_context(tc.tile_pool(name="s", bufs=3))
        qpool = ctx.enter_context(tc.tile_pool(name="q", bufs=6))
        ppool = ctx.enter_context(tc.tile_pool(name="pt", bufs=3, space="PSUM"))
        popool = ctx.enter_context(tc.tile_pool(name="po", bufs=3, space="PSUM"))

        identf = const.tile([128, 128], FP32)
        make_identity(nc, identf[:])
        ident = const.tile([128, 128], FP32R)
        nc.vector.tensor_copy(ident[:], identf[:])
        ones_rowf = const.tile([1, 128], FP32)
        nc.gpsimd.memset(ones_rowf[:], 1.0)
        ones_row = const.tile([1, 128], FP32R)
        nc.vector.tensor_copy(ones_row[:], ones_rowf[:])
        ones_colf = const.tile([128, 1], FP32)
        nc.gpsimd.memset(ones_colf[:], 1.0)
        ones_col = const.tile([128, 1], FP32R)
        nc.vector.tensor_copy(ones_col[:], ones_colf[:])

        # --- preprocessing: wT[f, ks, p] scaled by -2 (so PSUM = -2*cross
        # directly) and wsq_bcast = |w_p|^2 broadcast to all partitions ---
        w_sb = wpool.tile([128, PB, F], FP32R)  # [p_in, p_blk, f]
        # load w on the store ring so x-chunk loads start immediately on sync
        nc.scalar.dma_start(w_sb[:], w_r[:])
        wT = wpool.tile([128, KS, P], FP32R)  # [f_in, f_blk, p], holds -2*w^T
        for ks in range(KS):
            pt = ppool.tile([128, P], FP32R, tag="pt")
            for pb in range(PB):
                nc.tensor.transpose(
                    pt[:, pb * 128 : (pb + 1) * 128],
                    w_sb[:, pb, ks * 128 : (ks + 1) * 128],
                    ident[:],
                )
            nc.vector.tensor_scalar_mul(wT[:, ks, :], pt[:], -2.0)
        wsq_ps = popool.tile([1, P], FP32, tag="po")
        for ks in range(KS):
            s = spool.tile([128, P], FP32R, tag="s")
            nc.vector.tensor_mul(s[:], wT[:, ks, :], wT[:, ks, :])
            nc.tensor.matmul(
                wsq_ps[:],
                lhsT=ones_col[:],
                rhs=s[:],
                start=(ks == 0),
                stop=(ks == KS - 1),
            )
        # wsq_ps = sum_f (-2w)^2 = 4*|w|^2; scale by 0.25 on the way out
        wsq_row = const.tile([1, P], FP32R)
        nc.scalar.activation(wsq_row[:], wsq_ps[:], AF.Copy, scale=0.25)
        psb = popool.tile([128, P], FP32, tag="po")
        nc.tensor.matmul(psb[:], lhsT=ones_row[:], rhs=wsq_row[:], start=True, stop=True)
        wsq_bcast = const.tile([128, P], FP32)
        nc.vector.tensor_copy(wsq_bcast[:], psb[:])

        # --- main loop over row chunks ---
        for xr_c, or_c, csz in sched:
            xc = xpool.tile([128, csz, F], FP32R, tag="x")
            nc.sync.dma_start(xc[:], xr_c)
            oc = opool.tile([128, csz, P], FP32, tag="o")
            for t in range(csz):
                xv = xc[:, t, :]
                # row sums of squares on ACT (single Square+accum op)
                s = spool.tile([128, F], FP32, tag="s")
                xsq = qpool.tile([128, 1], FP32, tag="q")
                nc.scalar.activation(s[:], xv, AF.Square, accum_out=xsq[:])
                pt = ppool.tile([128, F], FP32R, tag="pt")
                for ks in range(KS):
                    nc.tensor.transpose(
                        pt[:, ks * 128 : (ks + 1) * 128],
                        xv[:, ks * 128 : (ks + 1) * 128],
                        ident[:],
                    )
                # PSUM->SBUF copy of xT: alternate DVE/ACT to balance load
                xT = xtpool.tile([128, F], FP32R, tag="xt")
                if t % 8 in (1, 4, 7):
                    nc.scalar.activation(xT[:], pt[:], AF.Copy)
                else:
                    nc.vector.tensor_copy(xT[:], pt[:])
                po = popool.tile([128, P], FP32, tag="po")
                for ks in range(KS):
                    nc.tensor.matmul(
                        po[:],
                        lhsT=xT[:, ks * 128 : (ks + 1) * 128],
                        rhs=wT[:, ks, :],
                        start=(ks == 0),
                        stop=(ks == KS - 1),
                    )
                # po = -2*cross; out = (po + xsq) + wsq  in one DVE op
                nc.vector.scalar_tensor_tensor(
                    oc[:, t, :], po[:], xsq[:], wsq_bcast[:], ALU.add, ALU.add
                )
                if csz >= 4 and t == csz // 2 - 1:
                    nc.scalar.dma_start(or_c[:, : csz // 2], oc[:, : csz // 2, :])
            if csz >= 4:
                nc.scalar.dma_start(or_c[:, csz // 2 :], oc[:, csz // 2 :, :])
            else:
                nc.scalar.dma_start(or_c[:], oc[:])

    split_multi_waits(nc)
    drop_sem_range_clear(nc)
    return nc


_CACHE: dict = {}


def kernel(inputs: np.ndarray, w: np.ndarray) -> np.ndarray:
    """Full-input entry point: shards rows across 8 NeuronCores, runs the
    Bass program SPMD, gathers the full [N, P] output."""
    from concourse.bass_utils import run_bass_kernel_spmd

    inputs = np.ascontiguousarray(np.asarray(inputs), dtype=np.float32)
    w = np.ascontiguousarray(np.asarray(w), dtype=np.float32)
    assert inputs.shape == (N, F) and w.shape == (P, F)

    if "nc" not in _CACHE:
        _CACHE["nc"] = build_bass()
    nc = _CACHE["nc"]

    shards = np.split(inputs, NCORES, axis=0)
    in_maps = [{"inputs": s, "w": w} for s in shards]
    res = run_bass_kernel_spmd(nc, in_maps, core_ids=list(range(NCORES)))
    return np.concatenate([r["out"] for r in res.results], axis=0)


# revision 32
# speedup vs baseline: 1.1595x; 1.1595x over previous
"""Trainium2 Bass/Tile kernel: squared-L2 distance to prototypes (vq codebook).

D[n, p] = |x_n|^2 + |w_p|^2 - 2 * x @ w^T
  x: [65536, 512] f32, w: [512, 512] f32 -> D: [65536, 512] f32

Sharding: x rows split 8 ways across NeuronCores (data parallel), w replicated.
Per-core work: [8192, 512] @ [512, 512]^T plus rank-1 row/col corrections.

Per-core pipeline (all static python loops, Tile framework), measured at
~122 us/core on TRN2 (8 cores in parallel; DMA-bound, ~330 GB/s aggregate):
  - preprocess: w -> wT scaled by -2 (PE transposes), wsq broadcast tile
  - per 128-row tile of x:
      ACT: xsq = row-sum of x^2 (Square activation + accumulator)
      PE : 4x 128x128 transposes of the x tile into PSUM (fp32r)
      ACT/DVE: copy PSUM -> SBUF (xT), alternating engines to balance
      PE : 4 accumulating matmuls vs -2*wT (fp32r) -> PSUM = -2*cross
      DVE: out = (psum + xsq) + wsq_bcast in one scalar_tensor_tensor op
      DMA: 2MB chunk loads (contiguous 16KB/partition descriptors via
           (c p t) row mapping), 1MB half-chunk stores on the second
           HWDGE ring

float32r is fp32 bit-layout run in the PE's fast (tf32-like) matmul mode:
1 cycle/row vs 4 for exact fp32. Accumulation stays fp32 in PSUM. Walrus
requires every producer feeding an fp32r matmul to emit fp32r-rounded
output, so x/w DRAM tensors and all PE-operand tiles are declared float32r
(external inputs and DVE/ACT/memset writes all count as rounded).
"""

from contextlib import ExitStack

import numpy as np

import concourse.bass as bass
import concourse.mybir as mybir
import concourse.tile as tile
from concourse.masks import make_identity


def split_multi_waits(nc: bass.Bass, limit: int = 1) -> int:
    """Walrus codegen on this stack encodes at most one sync-wait per TPB
    instruction ("Too many sync wait commands"). Hoist excess waits onto
    same-engine InstNoOp carriers inserted just before the instruction —
    engine queues run in order, so stalling the nop stalls the instruction."""
    n_split = 0
    for block in nc.m.functions[0].blocks:
        new_insts = []
        for inst in block.instructions:
            si = inst.sync_info
            if si is not None and si.on_wait and len(si.on_wait) > limit:
                waits = list(si.on_wait)
                extra, keep = waits[: len(waits) - limit], waits[len(waits) - limit :]
                for i in range(0, len(extra), limit):
                    nop = mybir.InstNoOp(
                        name=nc.get_next_instruction_name(),
                        engine=inst.engine,
                        ins=[],
                        outs=[],
                        sync_info=mybir.SyncInfo(
                            on_wait=extra[i : i + limit], on_update=[]
                        ),
                        bass_nofuse=True,
                    )
                    nc.register_instruction(nop)
                    new_insts.append(nop)
                    n_split += 1
                inst.sync_info = mybir.SyncInfo(
                    on_wait=keep, on_update=list(si.on_update)
                )
            new_insts.append(inst)
        block.instructions[:] = new_insts
    return n_split

N, F, P = 65536, 512, 512
NCORES = 8
N_LOC = N // NCORES  # 8192 rows per core

FP32 = mybir.dt.float32
FP32R = mybir.dt.float32r
AF = mybir.ActivationFunctionType
ALU = mybir.AluOpType

KS = F // 128  # 4 contraction slices
PB = P // 128  # 4 prototype row-blocks


def drop_sem_range_clear(nc: bass.Bass) -> int:
    """This stack's walrus rejects the EVENT_SEMAPHORE_RANGE_CLEAR InstISA
    ("ISA wrong length") that bass emits as kernel-tail semaphore cleanup.
    Drop it; re-execution correctness is validated by running twice."""
    n = 0
    for block in nc.m.functions[0].blocks:
        keep = []
        for inst in block.instructions:
            if (
                isinstance(inst, mybir.InstISA)
                and getattr(inst, "op_name", None) == "EVENT_SEMAPHORE_RANGE_CLEAR"
            ):
                n += 1
                continue
            keep.append(inst)
        block.instructions[:] = keep
    return n


def build_bass(n_loc: int = N_LOC, chunk: int = 8) -> bass.Bass:
    """Build the per-core Bass program. n_loc = rows of x on this core."""
    tiles = n_loc // 128
    assert tiles % chunk == 0
    nchunks = tiles // chunk

    nc = bass.Bass("TRN2", target_bir_lowering=False, debug=False)
    x = nc.dram_tensor("inputs", [n_loc, F], FP32R, kind="ExternalInput").ap()
    w = nc.dram_tensor("w", [P, F], FP32R, kind="ExternalInput").ap()
    out = nc.dram_tensor("out", [n_loc, P], FP32, kind="ExternalOutput").ap()

    # Partition p of chunk c holds rows [c*128*chunk + p*chunk, +chunk): each
    # partition's slice is `chunk` consecutive rows = one contiguous DRAM
    # extent (16KB at chunk=8), so the DMA emits 128 big descriptors instead
    # of 128*chunk 2KB ones. Row->tile mapping changes but rows are
    # independent, so compute is unaffected as long as out uses the same map.
    x_r = x.rearrange("(c p t) f -> c p t f", p=128, t=chunk)
    out_r = out.rearrange("(c p t) q -> c p t q", p=128, t=chunk)
    w_r = w.rearrange("(b p) f -> p b f", p=128)

    # chunk schedule: taper the first and last full chunks into half chunks
    # so the pipeline fill (first load -> first compute) and drain
    # (final compute + store) are both halved
    sched = [(x_r[c], out_r[c], chunk) for c in range(nchunks)]
    if nchunks >= 4 and chunk % 2 == 0:
        h = chunk // 2
        rows = 128 * chunk
        xh_r = x[:rows, :].rearrange("(c p t) f -> c p t f", p=128, t=h)
        oh_r = out[:rows, :].rearrange("(c p t) q -> c p t q", p=128, t=h)
        tail0 = (nchunks - 1) * rows
        xt_r = x[tail0:, :].rearrange("(c p t) f -> c p t f", p=128, t=h)
        ot_r = out[tail0:, :].rearrange("(c p t) q -> c p t q", p=128, t=h)
        sched = (
            [(xh_r[0], oh_r[0], h), (xh_r[1], oh_r[1], h)]
            + sched[1:-1]
            + [(xt_r[0], ot_r[0], h), (xt_r[1], ot_r[1], h)]
        )

    with tile.TileContext(nc) as tc, ExitStack() as ctx:
        const = ctx.enter_context(tc.tile_pool(name="const", bufs=1))
        wpool = ctx.enter_context(tc.tile_pool(name="w", bufs=1))
        xpool = ctx.enter_context(tc.tile_pool(name="x", bufs=3))
        opool = ctx.enter_context(tc.tile_pool(name="o", bufs=3))
        xtpool = ctx.enter_context(tc.tile_pool(name="xt", bufs=6))
        spool = ctx.enter_context(tc.tile_pool(name="s", bufs=3))
        qpool = ctx.enter_context(tc.tile_pool(name="q", bufs=6))
        ppool = ctx.enter_context(tc.tile_pool(name="pt", bufs=4, space="PSUM"))
        popool = ctx.enter_context(tc.tile_pool(name="po", bufs=3, space="PSUM"))

        identf = const.tile([128, 128], FP32)
        make_identity(nc, identf[:])
        ident = const.tile([128, 128], FP32R)
        nc.vector.tensor_copy(ident[:], identf[:])
        ones_rowf = const.tile([1, 128], FP32)
        nc.gpsimd.memset(ones_rowf[:], 1.0)
        ones_row = const.tile([1, 128], FP32R)
        nc.vector.tensor_copy(ones_row[:], ones_rowf[:])
        ones_colf = const.tile([128, 1], FP32)
        nc.gpsimd.memset(ones_colf[:], 1.0)
        ones_col = const.tile([128, 1], FP32R)
        nc.vector.tensor_copy(ones_col[:], ones_colf[:])

        # --- preprocessing: wT[f, ks, p] scaled by -2 (so PSUM = -2*cross
        # directly) and wsq_bcast = |w_p|^2 broadcast to all partitions ---
        w_sb = wpool.tile([128, PB, F], FP32R)  # [p_in, p_blk, f]
        nc.sync.dma_start(w_sb[:], w_r[:])
        wT = wpool.tile([128, KS, P], FP32R)  # [f_in, f_blk, p], holds -2*w^T
        for ks in range(KS):
            pt = ppool.tile([128, P], FP32R, tag="pt")
            for pb in range(PB):
                nc.tensor.transpose(
                    pt[:, pb * 128 : (pb + 1) * 128],
                    w_sb[:, pb, ks * 128 : (ks + 1) * 128],
                    ident[:],
                )
            nc.vector.tensor_scalar_mul(wT[:, ks, :], pt[:], -2.0)
        wsq_ps = popool.tile([1, P], FP32, tag="po")
        for ks in range(KS):
            s = spool.tile([128, P], FP32R, tag="s")
            nc.vector.tensor_mul(s[:], wT[:, ks, :], wT[:, ks, :])
            nc.tensor.matmul(
                wsq_ps[:],
                lhsT=ones_col[:],
                rhs=s[:],
                start=(ks == 0),
                stop=(ks == KS - 1),
            )
        # wsq_ps = sum_f (-2w)^2 = 4*|w|^2; scale by 0.25 on the way out
        wsq_row = const.tile([1, P], FP32R)
        nc.scalar.activation(wsq_row[:], wsq_ps[:], AF.Copy, scale=0.25)
        psb = popool.tile([128, P], FP32, tag="po")
        nc.tensor.matmul(psb[:], lhsT=ones_row[:], rhs=wsq_row[:], start=True, stop=True)
        wsq_bcast = const.tile([128, P], FP32)
        nc.vector.tensor_copy(wsq_bcast[:], psb[:])

        # --- main loop over row chunks ---
        for xr_c, or_c, csz in sched:
            xc = xpool.tile([128, csz, F], FP32R, tag="x")
            nc.sync.dma_start(xc[:], xr_c)
            oc = opool.tile([128, csz, P], FP32, tag="o")
            for t in range(csz):
                xv = xc[:, t, :]
                # row sums of squares on ACT (single Square+accum op)
                s = spool.tile([128, F], FP32, tag="s")
                xsq = qpool.tile([128, 1], FP32, tag="q")
                nc.scalar.activation(s[:], xv, AF.Square, accum_out=xsq[:])
                pt = ppool.tile([128, F], FP32R, tag="pt")
                for ks in range(KS):
                    nc.tensor.transpose(
                        pt[:, ks * 128 : (ks + 1) * 128],
                        xv[:, ks * 128 : (ks + 1) * 128],
                        ident[:],
                    )
                # PSUM->SBUF copy of xT: alternate DVE/ACT to balance load
                xT = xtpool.tile([128, F], FP32R, tag="xt")
                if t % 8 in (1, 4, 7):
                    nc.scalar.activation(xT[:], pt[:], AF.Copy)
                else:
                    nc.vector.tensor_copy(xT[:], pt[:])
                po = popool.tile([128, P], FP32, tag="po")
                for ks in range(KS):
                    nc.tensor.matmul(
                        po[:],
                        lhsT=xT[:, ks * 128 : (ks + 1) * 128],
                        rhs=wT[:, ks, :],
                        start=(ks == 0),
                        stop=(ks == KS - 1),
                    )
                # po = -2*cross; out = (po + xsq) + wsq  in one DVE op
                nc.vector.scalar_tensor_tensor(
                    oc[:, t, :], po[:], xsq[:], wsq_bcast[:], ALU.add, ALU.add
                )
                if csz >= 4 and t == csz // 2 - 1:
                    nc.scalar.dma_start(or_c[:, : csz // 2], oc[:, : csz // 2, :])
            if csz >= 4:
                nc.scalar.dma_start(or_c[:, csz // 2 :], oc[:, csz // 2 :, :])
            else:
                nc.scalar.dma_start(or_c[:], oc[:])

    split_multi_waits(nc)
    drop_sem_range_clear(nc)
    return nc


_CACHE: dict = {}


def kernel(inputs: np.ndarray, w: np.ndarray) -> np.ndarray:
    """Full-input entry point: shards rows across 8 NeuronCores, runs the
    Bass program SPMD, gathers the full [N, P] output."""
    from concourse.bass_utils import run_bass_kernel_spmd

    inputs = np.ascontiguousarray(np.asarray(inputs), dtype=np.float32)
    w = np.ascontiguousarray(np.asarray(w), dtype=np.float32)
    assert inputs.shape == (N, F) and w.shape == (P, F)

    if "nc" not in _CACHE:
        _CACHE["nc"] = build_bass()
    nc = _CACHE["nc"]

    shards = np.split(inputs, NCORES, axis=0)
    in_maps = [{"inputs": s, "w": w} for s in shards]
    res = run_bass_kernel_spmd(nc, in_maps, core_ids=list(range(NCORES)))
    return np.concatenate([r["out"] for r in res.results], axis=0)
